# revision 1
# baseline (speedup 1.0000x reference)
"""MinNormSolver kernel for 8 trn2 NeuronCores.

Strategy:
  - The only heavy op is the Gram matrix G = vecs @ vecs.T  ([16, 8M] f32).
  - Shard the feature dim across 8 cores (1M cols each).
  - Host packs each core's shard into a "block-transposed" layout so the
    TensorEngine can contract over the partition dim with full 128x128 tiles:
        X_s[p, b*16+i] = V[i, (s*8+b)*128 + p]
    One matmul  X_s.T @ X_s  accumulates 8 partial 16x16 Grams on the
    diagonal blocks of a [128,128] PSUM tile (off-diagonal blocks are
    garbage and ignored).
  - MODE=dr (default): fp8 DoubleRow perf mode contracts 256 features per
    matmul (2 k-tiles of 128), a 1.74x PE win vs normal mode (77ns per
    2048 features vs 67ns per 1024): tiles are [128, 2, 128] APs built via
    AP.rearrange from 2D slots, superblock = 2048 features, 489 MMs/core.
  - RAW=1 (default): hand-synced instruction stream instead of TileContext
    (kills Tile's exit semaphore-cleanup storm).  One semaphore per DMA
    tile: a shared cumulative counter is racy because the 16 per-engine
    increments of different DMAs can interleave.
  - DMA: uniform 1MB chunks (GS=32 superblocks, 8KB per-partition runs),
    BUFS=16 slots (16MB SBUF) for deep prefetch; one 512KB starter tile.
    Engines sustain ~26GB/s each (~354-390 GB/s/core aggregate), which is
    the binding constraint (PE needs 416 GB/s when unthrottled).
  - WARM=30 dummy matmuls on a zeroed scratch tile bridge the HAM
    clock-gate window (PE would otherwise run 1.2GHz for its first 3.4us).
  - Data is shipped as fp8e4m3: G ~ 8e6*I dominates and rounding noise is
    i.i.d., so the min-norm solution shifts by O(1e-4) relative only
    (measured 5.6e-5 vs the f32 reference).
  - The 250-iteration Frank-Wolfe solver runs on host (16x16 ops).
"""

import os
import sys

sys.path.insert(0, "/opt/trn_rl_repo")

import numpy as np

N_TASKS = 16
D_FEAT = 8_000_000
N_CORES = 8
P = 128                      # partitions per tile = contraction window
B = 8                        # 16-task chunks per superblock (M = B*16 = 128)
D_PER_CORE = D_FEAT // N_CORES          # 1_000_000

MODE = os.environ.get("MNS_MODE", "dr")      # "base" | "dr"
RAW = bool(int(os.environ.get("MNS_RAW", "1")))
KT = 2 if MODE == "dr" else 1                # k-tiles per matmul (DoubleRow)
SUPER_D = P * B * KT                         # features per superblock
S = -(-D_PER_CORE // SUPER_D)                # superblocks per core
D_PAD = S * SUPER_D
FREE = S * KT * P            # per-partition elements in the DRAM layout

DTYPE_STR = os.environ.get("MNS_DTYPE", "float8e4")
GS = int(os.environ.get("MNS_GS", "32"))
BUFS = int(os.environ.get("MNS_BUFS", "16"))
_DEF_RAMP = "8,8,16,16,32,32" if MODE == "base" else "16"
RAMP = [int(x) for x in os.environ.get("MNS_RAMP", _DEF_RAMP).split(",") if x]
ALT_DMA = bool(int(os.environ.get("MNS_ALT_DMA", "0")))
WARM = int(os.environ.get("MNS_WARM", "30"))   # dummy MMs to pre-warm HAM (raw mode)
REPS = int(os.environ.get("MNS_REPS", "1"))    # dev knob: HW reps, take min
BANKS = int(os.environ.get("MNS_BANKS", "1"))  # PSUM accumulation banks (raw mode)

_cache = {}


def _np_dtype():
    if DTYPE_STR == "float16":
        return np.float16
    import ml_dtypes

    return {
        "bfloat16": ml_dtypes.bfloat16,
        "float8e4": ml_dtypes.float8_e4m3,
        "float8e5": ml_dtypes.float8_e5m2,
    }[DTYPE_STR]


TAILQ = int(os.environ.get("MNS_TAILQ", "0"))  # fine-grained tail quantum


def _schedule():
    """(start_superblock, n_superblocks) DMA tiles; small tiles first so the
    PE starts within ~1-2us instead of waiting for a full mega-tile; with
    TAILQ, the last ~GS superblocks use fine tiles so the PE tail after the
    final byte is short."""
    sched = []
    s = 0
    for r in RAMP:
        if s + r > S:
            break
        sched.append((s, r))
        s += r
    tail_start = S - (GS + GS % TAILQ) if TAILQ else S
    while s < S:
        q = GS if s < tail_start else TAILQ
        gs = min(q, S - s)
        sched.append((s, gs))
        s += gs
    return sched

LAST_EXEC_NS = None


def _perf_mode(mybir):
    return mybir.MatmulPerfMode.DoubleRow if MODE == "dr" else None


def _build_nc_tile():
    import concourse.mybir as mybir
    from concourse import bacc, tile

    dt_in = getattr(mybir.dt, DTYPE_STR)
    pm = _perf_mode(mybir)
    nc = bacc.Bacc("TRN2", target_bir_lowering=False, debug=False, num_devices=N_CORES)
    h = nc.dram_tensor("h", [P, S * KT * P], dt_in, kind="ExternalInput")
    g = nc.dram_tensor("g", [P, P], mybir.dt.float32, kind="ExternalOutput")
    W = KT * P

    with tile.TileContext(nc) as tc:
        with (
            tc.tile_pool(name="inp", bufs=BUFS) as in_pool,
            tc.tile_pool(name="acc", bufs=1, space="PSUM") as psum_pool,
            tc.tile_pool(name="outp", bufs=1) as out_pool,
        ):
            acc = psum_pool.tile([P, P], mybir.dt.float32)
            for t, (s0, gs) in enumerate(_schedule()):
                mega = in_pool.tile([P, gs * W], dt_in, tag="mega")
                dma_eng = nc.scalar if (ALT_DMA and t % 2) else nc.sync
                dma_eng.dma_start(
                    mega[:, : gs * W], h[:, s0 * W : (s0 + gs) * W]
                )
                for k in range(gs):
                    s_idx = s0 + k
                    sb = mega[:, k * W : (k + 1) * W]
                    if MODE == "dr":
                        sb = sb.rearrange("p (t c) -> p t c", t=KT)
                    nc.tensor.matmul(
                        acc[:],
                        sb,
                        sb,
                        start=(s_idx == 0),
                        stop=(s_idx == S - 1),
                        perf_mode=pm,
                    )
            outt = out_pool.tile([P, P], mybir.dt.float32)
            nc.vector.tensor_copy(outt[:], acc[:])
            nc.sync.dma_start(g[:], outt[:])
    nc.finalize()
    return nc


def _build_nc_raw():
    """Hand-synced variant (no TileContext): linear DMA stream -> matmul
    stream -> copy -> out DMA, 3 semaphores.  Avoids Tile's entry/exit
    barriers and the ~200-semaphore cleanup storm."""
    import concourse.mybir as mybir
    from concourse import bacc
    from contextlib import ExitStack

    dt_in = getattr(mybir.dt, DTYPE_STR)
    pm = _perf_mode(mybir)
    nc = bacc.Bacc("TRN2", target_bir_lowering=False, debug=False, num_devices=N_CORES)
    h = nc.dram_tensor("h", [P, S * KT * P], dt_in, kind="ExternalInput")
    g = nc.dram_tensor("g", [P, P], mybir.dt.float32, kind="ExternalOutput")

    sched = _schedule()
    nt = len(sched)
    W = KT * P  # free-dim elements per superblock

    def _mm_ap(tensor2d, k):
        sb = tensor2d[:, k * W : (k + 1) * W]
        if MODE == "dr":
            sb = sb.rearrange("p (t c) -> p t c", t=KT)
        return sb

    with ExitStack() as ctx:
        slots = [
            ctx.enter_context(nc.sbuf_tensor(f"slot{i}", [P, GS * W], dt_in))
            for i in range(BUFS)
        ]
        warm = ctx.enter_context(nc.sbuf_tensor("warm", [P, W], dt_in))
        outt = ctx.enter_context(nc.sbuf_tensor("outt", [P, P], mybir.dt.float32))
        accs = [
            ctx.enter_context(nc.psum_tensor(f"accp{b}", [P, P], mybir.dt.float32))
            for b in range(BANKS)
        ]
        warmp = ctx.enter_context(nc.psum_tensor("warmp", [P, P], mybir.dt.float32))
        # One semaphore per DMA tile: a single shared counter would let a
        # mix of the 16 per-engine increments from different DMAs satisfy a
        # 16*(t+1) wait before tile t actually landed.
        dma_sems = [
            ctx.enter_context(nc.semaphore(f"dsem{t}")) for t in range(nt)
        ]
        gout_sem = ctx.enter_context(nc.semaphore("gout_sem"))
        pe_sem = ctx.enter_context(nc.semaphore("pe_sem"))
        out_sem = ctx.enter_context(nc.semaphore("out_sem"))
        warm_sem = ctx.enter_context(nc.semaphore("warm_sem"))
        block = ctx.enter_context(nc.Block())

        def _issue(eng, t, s0, gs):
            if t >= BUFS:
                eng.wait_ge(pe_sem, t - BUFS + 1)
            eng.dma_start(
                slots[t % BUFS][:, : gs * W],
                h[:, s0 * W : (s0 + gs) * W],
            ).then_inc(dma_sems[t], 16)

        @block.sync
        def _(sync):
            for t, (s0, gs) in enumerate(sched):
                if not (ALT_DMA and t % 2):
                    _issue(sync, t, s0, gs)
            sync.wait_ge(out_sem, 1)
            sync.dma_start(g[:], outt[:]).then_inc(gout_sem, 16)
            sync.wait_ge(gout_sem, 16)

        if ALT_DMA:
            @block.scalar
            def _(scalar):
                for t, (s0, gs) in enumerate(sched):
                    if t % 2:
                        _issue(scalar, t, s0, gs)

        @block.tensor
        def _(tensor):
            # HAM pre-warm: dummy matmuls keep the PE busy through the
            # clock-gate window while the first DMA lands.
            if WARM:
                tensor.wait_ge(warm_sem, 1)
                wap = _mm_ap(warm, 0)
                for _w in range(WARM):
                    nc.tensor.matmul(
                        warmp[:], wap, wap, start=True, stop=True, perf_mode=pm,
                        skip_group_check=True,
                    )
            for t, (s0, gs) in enumerate(sched):
                tensor.wait_ge(dma_sems[t], 16)
                mm = None
                for k in range(gs):
                    s_idx = s0 + k
                    sb = _mm_ap(slots[t % BUFS], k)
                    mm = nc.tensor.matmul(
                        accs[s_idx % BANKS][:],
                        sb,
                        sb,
                        start=(s_idx < BANKS),
                        stop=(s_idx >= S - BANKS),
                        perf_mode=pm,
                    )
                mm.then_inc(pe_sem, 1)

        @block.vector
        def _(vector):
            if WARM:
                nc.vector.memset(warm[:], 0).then_inc(warm_sem, 1)
            vector.wait_ge(pe_sem, nt)
            if BANKS == 1:
                nc.vector.tensor_copy(outt[:], accs[0][:]).then_inc(out_sem, 1)
            else:
                assert BANKS == 2
                nc.vector.tensor_tensor(
                    outt[:], accs[0][:], accs[1][:], mybir.AluOpType.add
                ).then_inc(out_sem, 1)

    nc.finalize()
    return nc


def _get_nc():
    if "nc" not in _cache:
        _cache["nc"] = _build_nc_raw() if RAW else _build_nc_tile()
    return _cache["nc"]


def _pack_core(v16, c):
    """v16: [16, D_FEAT] narrowed dtype.  Returns [P, S*KT*P] contiguous
    for core c.  Within a superblock the free dim is [t, b*16+i] per the
    feature map d = s*SUPER_D + b*(KT*P) + t*P + p."""
    shard = v16[:, c * D_PER_CORE : (c + 1) * D_PER_CORE]
    padded = np.zeros((N_TASKS, D_PAD), dtype=v16.dtype)
    padded[:, :D_PER_CORE] = shard
    # [16, S, B, KT, P] -> [P, S, KT, B, 16] -> [P, S*KT*P]
    out = np.ascontiguousarray(
        padded.reshape(N_TASKS, S, B, KT, P).transpose(4, 1, 3, 2, 0)
    ).reshape(P, S * KT * P)
    return out


def _line_solver(v11, v12, v22):
    EPS = 1e-8
    gamma0 = (v22 - v12) / (v11 + v22 - 2.0 * v12 + EPS)
    cost0 = v22 + gamma0 * (v12 - v22)
    gamma = np.where(v12 >= v11, 1.0, np.where(v12 >= v22, 0.0, gamma0))
    cost = np.where(v12 >= v11, v11, np.where(v12 >= v22, v22, cost0))
    return gamma, cost


def _solve_fw(G):
    """Replicates reference() given the [16,16] Gram matrix (float64)."""
    n = N_TASKS
    T_EPS = 1e-7
    STOP_CRIT = 1e-6
    MAX_ITER = 250
    i_triu, j_triu = np.triu_indices(n, 1)
    vivj = G[i_triu, j_triu]
    vivi = G[i_triu, i_triu]
    vjvj = G[j_triu, j_triu]
    gamma_p, cost_p = _line_solver(vivi, vivj, vjvj)
    off = int(np.argmin(cost_p))
    sol = np.zeros(n, dtype=G.dtype)
    sol[i_triu[off]] = gamma_p[off]
    sol[j_triu[off]] = 1.0 - gamma_p[off]
    igrid = np.arange(1, n + 1, dtype=G.dtype)

    for _ in range(MAX_ITER):
        s = sol
        grad = -(G @ s)
        # _next_point
        pg = grad - grad.sum() / n
        pg_safe = np.where(pg == 0.0, 1.0, pg)
        tm1 = -s / pg_safe
        tm2 = (1.0 - s) / pg_safe
        m1 = (pg < 0.0) & (tm1 > T_EPS)
        m2 = (pg > 0.0) & (tm2 > T_EPS)
        t = np.where(m1, tm1, np.inf).min() if m1.any() else 1.0
        if m2.any():
            t = min(t, np.where(m2, tm2, np.inf).min())
        gpt = pg * t + s
        # _proj_simplex
        srt = np.sort(gpt)[::-1]
        tmax = (np.cumsum(srt) - 1.0) / igrid
        cond = tmax[:-1] > srt[1:]
        tmax_f = tmax[:-1][np.argmax(cond)] if cond.any() else tmax[-1]
        new_pt = np.maximum(gpt - tmax_f, 0.0)

        Gs = G @ s
        Gn = G @ new_pt
        v11 = s @ Gs
        v12 = s @ Gn
        v22 = new_pt @ Gn
        gam, _ = _line_solver(v11, v12, v22)
        new_s = gam * s + (1.0 - gam) * new_pt
        if np.abs(new_s - s).sum() < STOP_CRIT:
            break  # reference freezes at the pre-update value
        sol = new_s
    return sol


def _extract_partial(psum_out):
    """Sum the 8 diagonal 16x16 blocks of the [128,128] per-core output."""
    blocks = psum_out.reshape(B, N_TASKS, B, N_TASKS)
    return sum(
        blocks[b, :, b, :].astype(np.float64) for b in range(B)
    )


def kernel(vecs):
    global LAST_EXEC_NS
    from concourse.bass_utils import run_bass_kernel_spmd

    vecs = np.asarray(vecs)
    assert vecs.shape == (N_TASKS, D_FEAT)
    v16 = vecs.astype(_np_dtype())

    in_maps = [{"h": _pack_core(v16, c)} for c in range(N_CORES)]

    nc = _get_nc()
    trace = bool(int(os.environ.get("MNS_TRACE", "0")))
    times = []
    for _ in range(REPS):
        res = run_bass_kernel_spmd(
            nc, in_maps, core_ids=list(range(N_CORES)), trace=trace
        )
        times.append(res.exec_time_ns)
    if REPS > 1:
        print("rep exec times:", times)
    LAST_EXEC_NS = min(t for t in times if t is not None) if any(times) else None
    _cache["last_results"] = res

    G = np.zeros((N_TASKS, N_TASKS), dtype=np.float64)
    for c in range(N_CORES):
        G += _extract_partial(np.asarray(res.results[c]["g"]))

    sol = _solve_fw(G)
    return sol.astype(np.float32)



# revision 4
# speedup vs baseline: 1.0263x; 1.0263x over previous
"""MinNormSolver kernel for 8 trn2 NeuronCores.

Strategy:
  - The only heavy op is the Gram matrix G = vecs @ vecs.T  ([16, 8M] f32).
  - Shard the feature dim across 8 cores (1M cols each).
  - Host packs each core's shard into a "block-transposed" layout so the
    TensorEngine can contract over the partition dim with full 128x128 tiles:
        X_s[p, b*16+i] = V[i, (s*8+b)*128 + p]
    One matmul  X_s.T @ X_s  accumulates 8 partial 16x16 Grams on the
    diagonal blocks of a [128,128] PSUM tile (off-diagonal blocks are
    garbage and ignored).
  - fp8 DoubleRow perf mode contracts 256 features per matmul (2 k-tiles
    of 128): tiles are [128, 2, 128] APs built via AP.rearrange, superblock
    = 2048 features, 489 MMs/core at ~78ns cadence (LDWEIGHTS-paced).
  - The FULL 16MB per-core input fits in SBUF (122.25KB of ~208KB usable
    per partition), so there is no slot recycling: one [128, 125184] fp8
    SBUF tensor, DMA'd in chunks, each chunk consumed by the PE as soon
    as its semaphore fires.
  - DMA chunks alternate between the two HWDGE queues (sync=SP,
    scalar=Activation).  ONE cumulative semaphore per queue: each engine
    drains its per-queue ring FIFO and increments once per instruction,
    so sem >= 16*(t+1) can only be satisfied when every engine finished
    instruction t (sum = 16*(t+1) with per-engine count <= t+1 forces
    all counts = t+1).  6 semaphores total (vs 20 in the tile-per-sem
    variant) to shrink the NEFF-exit semaphore-teardown storm.
  - Primer DMAs (one small dma_start per queue into a scratch tile) run
    first to absorb the SDMA engines' first-descriptor ramp (~0.9-3us,
    worst on engine 15 under profiling) while the PE does WARM dummy
    matmuls to bridge the HAM clock-gate window (PE runs 1.2GHz for its
    first ~3.4us otherwise).
  - Schedule: fine ramp chunks first (PE starts early), 32-superblock
    (1MB) chunks in the middle, fine tail chunks so the PE finishes
    right after the last byte lands.
  - Data is shipped as fp8e4m3: G ~ 8e6*I dominates and rounding noise is
    i.i.d. (measured 5.6e-5 rel err vs the f32 reference).
  - The 250-iteration Frank-Wolfe solver runs on host (16x16 ops).
"""

import os
import sys

sys.path.insert(0, "/opt/trn_rl_repo")

import numpy as np

N_TASKS = 16
D_FEAT = 8_000_000
N_CORES = 8
P = 128                      # partitions per tile = contraction window
B = 8                        # 16-task chunks per superblock (M = B*16 = 128)
D_PER_CORE = D_FEAT // N_CORES          # 1_000_000

KT = 2                                   # k-tiles per matmul (DoubleRow)
SUPER_D = P * B * KT                     # features per superblock (2048)
S = -(-D_PER_CORE // SUPER_D)            # superblocks per core (489)
D_PAD = S * SUPER_D
W = KT * P                               # free-dim elements per superblock
FREE = S * W                             # per-partition elements in DRAM layout

DTYPE_STR = os.environ.get("MNS_DTYPE", "float8e4")
GS = int(os.environ.get("MNS_GS", "32"))             # mid chunk superblocks
RAMP = [int(x) for x in os.environ.get("MNS_RAMP", "4,8,16,32").split(",") if x]
TAIL = [int(x) for x in os.environ.get("MNS_TAIL", "8,8,8,8,8,8,8,8").split(",") if x]
WARM = int(os.environ.get("MNS_WARM", "40"))         # dummy MMs to pre-warm HAM
# Warm-fill MMs inserted BEFORE waiting on chunk c's semaphore: keeps the PE
# busy through early DMA supply gaps so the HAM clock-gate never drops the
# PE back to 1.2GHz ("c:count,c:count").
FILLS = {
    int(k): int(v)
    for k, v in (
        kv.split(":")
        for kv in os.environ.get("MNS_FILLS", "1:6,2:40,3:20,4:6").split(",")
        if kv
    )
}
PRIME_SB = int(os.environ.get("MNS_PRIME", "8"))     # primer size (superblocks)
NQ = int(os.environ.get("MNS_NQ", "2"))              # 1=sync only, 2=sync+scalar
REPS = int(os.environ.get("MNS_REPS", "1"))          # dev knob: HW reps, take min

_cache = {}


def _np_dtype():
    if DTYPE_STR == "float16":
        return np.float16
    import ml_dtypes

    return {
        "bfloat16": ml_dtypes.bfloat16,
        "float8e4": ml_dtypes.float8_e4m3,
        "float8e5": ml_dtypes.float8_e5m2,
    }[DTYPE_STR]


def _schedule():
    """(start_superblock, n_superblocks) chunks: ramp, mid, tail."""
    tail_n = sum(TAIL)
    sched = []
    s = 0
    for r in RAMP:
        if s + r > S - tail_n:
            break
        sched.append((s, r))
        s += r
    while s < S - tail_n:
        gs = min(GS, S - tail_n - s)
        sched.append((s, gs))
        s += gs
    for t in TAIL:
        if s >= S:
            break
        gs = min(t, S - s)
        sched.append((s, gs))
        s += gs
    assert sum(n for _, n in sched) == S
    return sched


LAST_EXEC_NS = None


def _build_nc():
    """Hand-synced raw kernel: full-SBUF input, 2 HWDGE queues with one
    cumulative semaphore each, primer DMAs, warm MMs, fine ramp+tail."""
    import concourse.mybir as mybir
    from concourse import bacc
    from contextlib import ExitStack

    dt_in = getattr(mybir.dt, DTYPE_STR)
    pm = mybir.MatmulPerfMode.DoubleRow
    nc = bacc.Bacc("TRN2", target_bir_lowering=False, debug=False, num_devices=N_CORES)
    h = nc.dram_tensor("h", [P, FREE], dt_in, kind="ExternalInput")
    g = nc.dram_tensor("g", [P, P], mybir.dt.float32, kind="ExternalOutput")

    sched = _schedule()
    # queue id per chunk (round-robin), and per-queue instruction index
    # (primer = index 0 on each queue).
    qid = [c % NQ for c in range(len(sched))]
    qidx = []
    counts = [1] * NQ  # primers occupy slot 0
    for c in range(len(sched)):
        qidx.append(counts[qid[c]])
        counts[qid[c]] += 1

    def _mm_ap(tensor2d, k):
        sb = tensor2d[:, k * W : (k + 1) * W]
        return sb.rearrange("p (t c) -> p t c", t=KT)

    with ExitStack() as ctx:
        X = ctx.enter_context(nc.sbuf_tensor("X", [P, FREE], dt_in))
        warm = ctx.enter_context(nc.sbuf_tensor("warm", [P, W], dt_in))
        prime = ctx.enter_context(
            nc.sbuf_tensor("prime", [P, NQ * PRIME_SB * W], dt_in)
        )
        outt = ctx.enter_context(nc.sbuf_tensor("outt", [P, P], mybir.dt.float32))
        acc = ctx.enter_context(nc.psum_tensor("accp", [P, P], mybir.dt.float32))
        warmp = ctx.enter_context(nc.psum_tensor("warmp", [P, P], mybir.dt.float32))
        qsems = [ctx.enter_context(nc.semaphore(f"qsem{q}")) for q in range(NQ)]
        warm_sem = ctx.enter_context(nc.semaphore("warm_sem"))
        pe_sem = ctx.enter_context(nc.semaphore("pe_sem"))
        out_sem = ctx.enter_context(nc.semaphore("out_sem"))
        gout_sem = ctx.enter_context(nc.semaphore("gout_sem"))
        block = ctx.enter_context(nc.Block())

        def _issue_queue(eng, q):
            # primer: absorb first-descriptor / engine-ramp latency
            pw = PRIME_SB * W
            eng.dma_start(
                prime[:, q * pw : (q + 1) * pw], h[:, :pw]
            ).then_inc(qsems[q], 16)
            for c, (s0, gs) in enumerate(sched):
                if qid[c] == q:
                    eng.dma_start(
                        X[:, s0 * W : (s0 + gs) * W],
                        h[:, s0 * W : (s0 + gs) * W],
                    ).then_inc(qsems[q], 16)

        @block.sync
        def _(sync):
            _issue_queue(sync, 0)
            sync.wait_ge(out_sem, 1)
            sync.dma_start(g[:], outt[:]).then_inc(gout_sem, 16)
            sync.wait_ge(gout_sem, 16)

        if NQ > 1:
            @block.scalar
            def _(scalar):
                _issue_queue(scalar, 1)

        @block.gpsimd
        def _(gpsimd):
            nc.gpsimd.memset(warm[:], 0).then_inc(warm_sem, 1)

        @block.tensor
        def _(tensor):
            wap = _mm_ap(warm, 0)

            def _warm_mms(n):
                for _w in range(n):
                    nc.tensor.matmul(
                        warmp[:], wap, wap, start=True, stop=True, perf_mode=pm,
                        skip_group_check=True,
                    )

            if WARM:
                tensor.wait_ge(warm_sem, 1)
                _warm_mms(WARM)
            mm = None
            for c, (s0, gs) in enumerate(sched):
                _warm_mms(FILLS.get(c, 0))
                tensor.wait_ge(qsems[qid[c]], 16 * (qidx[c] + 1))
                for k in range(gs):
                    s_idx = s0 + k
                    sb = _mm_ap(X, s_idx)
                    mm = nc.tensor.matmul(
                        acc[:],
                        sb,
                        sb,
                        start=(s_idx == 0),
                        stop=(s_idx == S - 1),
                        perf_mode=pm,
                    )
            mm.then_inc(pe_sem, 1)

        @block.vector
        def _(vector):
            # guard: all input DMA complete (redundant with pe_sem, cheap)
            for q in range(NQ):
                vector.wait_ge(qsems[q], 16 * counts[q])
            vector.wait_ge(pe_sem, 1)
            nc.vector.tensor_copy(outt[:], acc[:]).then_inc(out_sem, 1)

    nc.finalize()
    return nc


def _get_nc():
    if "nc" not in _cache:
        _cache["nc"] = _build_nc()
    return _cache["nc"]


def _pack_core(v16, c):
    """v16: [16, D_FEAT] narrowed dtype.  Returns [P, S*KT*P] contiguous
    for core c.  Within a superblock the free dim is [t, b*16+i] per the
    feature map d = s*SUPER_D + b*(KT*P) + t*P + p."""
    shard = v16[:, c * D_PER_CORE : (c + 1) * D_PER_CORE]
    padded = np.zeros((N_TASKS, D_PAD), dtype=v16.dtype)
    padded[:, :D_PER_CORE] = shard
    # [16, S, B, KT, P] -> [P, S, KT, B, 16] -> [P, S*KT*P]
    out = np.ascontiguousarray(
        padded.reshape(N_TASKS, S, B, KT, P).transpose(4, 1, 3, 2, 0)
    ).reshape(P, S * KT * P)
    return out


def _line_solver(v11, v12, v22):
    EPS = 1e-8
    gamma0 = (v22 - v12) / (v11 + v22 - 2.0 * v12 + EPS)
    cost0 = v22 + gamma0 * (v12 - v22)
    gamma = np.where(v12 >= v11, 1.0, np.where(v12 >= v22, 0.0, gamma0))
    cost = np.where(v12 >= v11, v11, np.where(v12 >= v22, v22, cost0))
    return gamma, cost


def _solve_fw(G):
    """Replicates reference() given the [16,16] Gram matrix (float64)."""
    n = N_TASKS
    T_EPS = 1e-7
    STOP_CRIT = 1e-6
    MAX_ITER = 250
    i_triu, j_triu = np.triu_indices(n, 1)
    vivj = G[i_triu, j_triu]
    vivi = G[i_triu, i_triu]
    vjvj = G[j_triu, j_triu]
    gamma_p, cost_p = _line_solver(vivi, vivj, vjvj)
    off = int(np.argmin(cost_p))
    sol = np.zeros(n, dtype=G.dtype)
    sol[i_triu[off]] = gamma_p[off]
    sol[j_triu[off]] = 1.0 - gamma_p[off]
    igrid = np.arange(1, n + 1, dtype=G.dtype)

    for _ in range(MAX_ITER):
        s = sol
        grad = -(G @ s)
        # _next_point
        pg = grad - grad.sum() / n
        pg_safe = np.where(pg == 0.0, 1.0, pg)
        tm1 = -s / pg_safe
        tm2 = (1.0 - s) / pg_safe
        m1 = (pg < 0.0) & (tm1 > T_EPS)
        m2 = (pg > 0.0) & (tm2 > T_EPS)
        t = np.where(m1, tm1, np.inf).min() if m1.any() else 1.0
        if m2.any():
            t = min(t, np.where(m2, tm2, np.inf).min())
        gpt = pg * t + s
        # _proj_simplex
        srt = np.sort(gpt)[::-1]
        tmax = (np.cumsum(srt) - 1.0) / igrid
        cond = tmax[:-1] > srt[1:]
        tmax_f = tmax[:-1][np.argmax(cond)] if cond.any() else tmax[-1]
        new_pt = np.maximum(gpt - tmax_f, 0.0)

        Gs = G @ s
        Gn = G @ new_pt
        v11 = s @ Gs
        v12 = s @ Gn
        v22 = new_pt @ Gn
        gam, _ = _line_solver(v11, v12, v22)
        new_s = gam * s + (1.0 - gam) * new_pt
        if np.abs(new_s - s).sum() < STOP_CRIT:
            break  # reference freezes at the pre-update value
        sol = new_s
    return sol


def _extract_partial(psum_out):
    """Sum the 8 diagonal 16x16 blocks of the [128,128] per-core output."""
    blocks = psum_out.reshape(B, N_TASKS, B, N_TASKS)
    return sum(
        blocks[b, :, b, :].astype(np.float64) for b in range(B)
    )


def kernel(vecs):
    global LAST_EXEC_NS
    from concourse.bass_utils import run_bass_kernel_spmd

    vecs = np.asarray(vecs)
    assert vecs.shape == (N_TASKS, D_FEAT)
    v16 = vecs.astype(_np_dtype())

    in_maps = [{"h": _pack_core(v16, c)} for c in range(N_CORES)]

    nc = _get_nc()
    trace = bool(int(os.environ.get("MNS_TRACE", "0")))
    times = []
    for _ in range(REPS):
        res = run_bass_kernel_spmd(
            nc, in_maps, core_ids=list(range(N_CORES)), trace=trace
        )
        times.append(res.exec_time_ns)
    if REPS > 1:
        print("rep exec times:", times)
    LAST_EXEC_NS = min(t for t in times if t is not None) if any(times) else None
    _cache["last_results"] = res

    G = np.zeros((N_TASKS, N_TASKS), dtype=np.float64)
    for c in range(N_CORES):
        G += _extract_partial(np.asarray(res.results[c]["g"]))

    sol = _solve_fw(G)
    return sol.astype(np.float32)
